# revision 17
# baseline (speedup 1.0000x reference)
"""Trainium2 Bass kernel for the grid min-plus DP layer.

Problem: images [512, 1, 256, 256] f32.
  theta = softplus(images[:, 0])
  V[i,j] = theta[i,j] + min(V[i-1,j], V[i-1,j-1], V[i,j-1])   (V[-1,*] = +inf)
  output = V[H-1, W-1] per image, shape [512].

Strategy: pure data parallel over 8 NeuronCores (64 images per core).
Per core, images live on SBUF partitions (64 partitions), W on the free
dim.  Each DP row is exactly a first-order recurrence
    state = min(A[j], state) + theta[j],   A[j] = min(V[j], V[j-1])
which is one DVE tensor_tensor_scan(op0=min, op1=add) per row plus one
DVE tensor_tensor(min) for A.  Softplus runs on the scalar (ACT) engine,
theta streams from DRAM in 8-row chunks, all overlapped with the DVE
scan chain.
"""

import sys

import numpy as np

if "/opt/trn_rl_repo" not in sys.path:
    sys.path.insert(0, "/opt/trn_rl_repo")

import concourse.bacc as bacc
import concourse.bass as bass
import concourse.mybir as mybir
from concourse.bass_utils import run_bass_kernel_spmd
from concourse.tile import TileContext

B, H, W = 512, 256, 256
N_CORES = 8
BL = B // N_CORES  # 64 images per core
BIG = 1e30         # finite stand-in for +inf (matches reference NULL)
ROWS_PER_CHUNK = 8

_cache = {}


def build(h=H, softplus_mode="exp_ln"):
    """Build the single-core Bass program (SPMD across 8 cores).

    softplus_mode: "exp_ln" computes softplus as ln(exp(x) + 1) on ACT
    (the native Softplus ACT func has no function-set in this compiler).
    """
    nc = bacc.Bacc("TRN2", target_bir_lowering=False, debug=False, num_devices=N_CORES)
    img = nc.declare_dram_parameter("images", [BL, h * W], mybir.dt.float32, isOutput=False)
    outp = nc.declare_dram_parameter("out", [BL, 1], mybir.dt.float32, isOutput=True)

    with TileContext(nc) as tc:
        with tc.tile_pool(name="const", bufs=1) as cpool, \
             tc.tile_pool(name="ld", bufs=8) as ldpool, \
             tc.tile_pool(name="work", bufs=3) as wpool, \
             tc.tile_pool(name="row", bufs=2) as rpool:
            # vpad[:, 0] = BIG forever; vpad[:, 1:W+1] = current row's V.
            vpad = cpool.tile([BL, W + 1], mybir.dt.float32)
            big = cpool.tile([BL, W], mybir.dt.float32)
            sink = cpool.tile([BL, 1], mybir.dt.float32)
            nc.vector.memset(vpad[:, 0:1], BIG)
            nc.vector.memset(big[:, :], BIG)

            n_chunks = h // ROWS_PER_CHUNK
            for c in range(n_chunks):
                # bufs=8 matches the 8 HWDGE lanes Tile round-robins DMAs
                # over: slot reuse (c-8) lands on the SAME lane, making the
                # WAW ordering implicit (FIFO) so each load carries only the
                # single WAR wait the 1-wait-slot DMA ISA format allows.
                raw = ldpool.tile([BL, ROWS_PER_CHUNK * W], mybir.dt.float32, tag="raw")
                nc.sync.dma_start(
                    out=raw[:, :],
                    in_=img[:, c * ROWS_PER_CHUNK * W:(c + 1) * ROWS_PER_CHUNK * W],
                )
                th = wpool.tile([BL, ROWS_PER_CHUNK * W], mybir.dt.float32, tag="th")
                if softplus_mode == "softplus":
                    nc.scalar.activation(th[:, :], raw[:, :], mybir.ActivationFunctionType.Softplus)
                else:
                    # Two ACT ops with separate output tiles: the Activation
                    # ISA format has a single sync-wait slot, so each op may
                    # carry at most one cross-engine dependency (Exp: DMA
                    # raw-ready; Ln: DVE th-slot release).
                    e = wpool.tile([BL, ROWS_PER_CHUNK * W], mybir.dt.float32, tag="e")
                    nc.scalar.activation(e[:, :], raw[:, :], mybir.ActivationFunctionType.Exp)
                    nc.scalar.activation(th[:, :], e[:, :], mybir.ActivationFunctionType.Ln, bias=1.0)

                # The scan ISA format has very few semaphore-wait slots;
                # absorb the ACT->DVE dependency on this chunk into a
                # normal-format DVE op so scans carry no cross-engine waits.
                nc.vector.tensor_copy(sink[:, :], th[:, 0:1])

                for rr in range(ROWS_PER_CHUNK):
                    r = c * ROWS_PER_CHUNK + rr
                    th_row = th[:, rr * W:(rr + 1) * W]
                    if r == 0:
                        # First row: plain cumsum.  A=BIG, initial=0:
                        # state = min(BIG, state) + th = state + th.
                        nc.vector.tensor_tensor_scan(
                            vpad[:, 1:W + 1], big[:, :], th_row, 0.0,
                            mybir.AluOpType.min, mybir.AluOpType.add,
                        )
                    else:
                        # A[j] = min(V[j], V[j-1])  (vpad[:,0] = BIG handles j=0)
                        a = rpool.tile([BL, W], mybir.dt.float32, tag="A")
                        nc.vector.tensor_tensor(
                            a[:, :], vpad[:, 1:W + 1], vpad[:, 0:W], mybir.AluOpType.min,
                        )
                        nc.vector.tensor_tensor_scan(
                            vpad[:, 1:W + 1], a[:, :], th_row, BIG,
                            mybir.AluOpType.min, mybir.AluOpType.add,
                        )

            nc.sync.dma_start(out=outp[:, :], in_=vpad[:, W:W + 1])

    nc.compile()  # bacc legalization (incl. 1-wait-per-inst event-sem split)
    return nc


def run(images, trace=False):
    """images: np [512, 1, 256, 256] f32 -> (out [512] f32, BassKernelResults)."""
    images = np.ascontiguousarray(np.asarray(images, dtype=np.float32))
    key = "nc"
    if key not in _cache:
        _cache[key] = build()
    nc = _cache[key]
    shards = images.reshape(N_CORES, BL, H * W)
    in_maps = [{"images": np.ascontiguousarray(shards[i])} for i in range(N_CORES)]
    res = run_bass_kernel_spmd(nc, in_maps, list(range(N_CORES)), trace=trace)
    outs = [np.asarray(r["out"]).reshape(BL) for r in res.results]
    return np.concatenate(outs).astype(np.float32), res


def kernel(images):
    out, _ = run(images, trace=False)
    return out


# revision 18
# speedup vs baseline: 27.0640x; 27.0640x over previous
"""Trainium2 Bass kernel for the grid min-plus DP layer.

Problem: images [512, 1, 256, 256] f32.
  theta = softplus(images[:, 0])
  V[i,j] = theta[i,j] + min(V[i-1,j], V[i-1,j-1], V[i,j-1])   (V[-1,*] = +inf)
  output = V[H-1, W-1] per image, shape [512].

Strategy: pure data parallel over 8 NeuronCores (64 images per core).
Per core, images live on SBUF partitions (64 partitions), W on the free
dim.  Each DP row is exactly a first-order recurrence
    state = min(A[j], state) + theta[j],   A[j] = min(V[j], V[j-1])
which is one DVE tensor_tensor_scan(op0=min, op1=add) per row plus one
DVE tensor_tensor(min) for A.  Softplus runs on the scalar (ACT) engine,
theta streams from DRAM in 8-row chunks, all overlapped with the DVE
scan chain.
"""

import sys

import numpy as np

if "/opt/trn_rl_repo" not in sys.path:
    sys.path.insert(0, "/opt/trn_rl_repo")

import concourse.bacc as bacc
import concourse.bass as bass
import concourse.mybir as mybir
from concourse.bass_utils import run_bass_kernel_spmd
from concourse.tile import TileContext

B, H, W = 512, 256, 256
N_CORES = 8
BL = B // N_CORES  # 64 images per core
BIG = 1e30         # finite stand-in for +inf (matches reference NULL)
ROWS_PER_CHUNK = 8

_cache = {}


def build(h=H, softplus_mode="exp_ln", repeat=1):
    """Build the single-core Bass program (SPMD across 8 cores).

    softplus_mode: "exp_ln" computes softplus as ln(exp(x) + 1) on ACT
    (the native Softplus ACT func has no function-set in this compiler).
    repeat: run the whole DP `repeat` times back-to-back (benchmarking:
    the slope over repeat isolates device time from dispatch overhead).
    """
    nc = bacc.Bacc("TRN2", target_bir_lowering=False, debug=False, num_devices=N_CORES)
    img = nc.declare_dram_parameter("images", [BL, h * W], mybir.dt.float32, isOutput=False)
    outp = nc.declare_dram_parameter("out", [BL, 1], mybir.dt.float32, isOutput=True)

    with TileContext(nc) as tc:
        with tc.tile_pool(name="const", bufs=1) as cpool, \
             tc.tile_pool(name="ld", bufs=8) as ldpool, \
             tc.tile_pool(name="work", bufs=3) as wpool, \
             tc.tile_pool(name="row", bufs=2) as rpool:
            # vpad[:, 0] = BIG forever; vpad[:, 1:W+1] = current row's V.
            vpad = cpool.tile([BL, W + 1], mybir.dt.float32)
            big = cpool.tile([BL, W], mybir.dt.float32)
            sink = cpool.tile([BL, 1], mybir.dt.float32)
            nc.vector.memset(vpad[:, 0:1], BIG)
            nc.vector.memset(big[:, :], BIG)

            n_chunks = h // ROWS_PER_CHUNK
            for c in range(n_chunks * repeat):
                c = c % n_chunks
                # bufs=8 matches the 8 HWDGE lanes Tile round-robins DMAs
                # over: slot reuse (c-8) lands on the SAME lane, making the
                # WAW ordering implicit (FIFO) so each load carries only the
                # single WAR wait the 1-wait-slot DMA ISA format allows.
                raw = ldpool.tile([BL, ROWS_PER_CHUNK * W], mybir.dt.float32, tag="raw")
                nc.sync.dma_start(
                    out=raw[:, :],
                    in_=img[:, c * ROWS_PER_CHUNK * W:(c + 1) * ROWS_PER_CHUNK * W],
                )
                th = wpool.tile([BL, ROWS_PER_CHUNK * W], mybir.dt.float32, tag="th")
                if softplus_mode == "softplus":
                    nc.scalar.activation(th[:, :], raw[:, :], mybir.ActivationFunctionType.Softplus)
                else:
                    # Two ACT ops with separate output tiles: the Activation
                    # ISA format has a single sync-wait slot, so each op may
                    # carry at most one cross-engine dependency (Exp: DMA
                    # raw-ready; Ln: DVE th-slot release).
                    e = wpool.tile([BL, ROWS_PER_CHUNK * W], mybir.dt.float32, tag="e")
                    nc.scalar.activation(e[:, :], raw[:, :], mybir.ActivationFunctionType.Exp)
                    nc.scalar.activation(th[:, :], e[:, :], mybir.ActivationFunctionType.Ln, bias=1.0)

                # The scan ISA format has very few semaphore-wait slots;
                # absorb the ACT->DVE dependency on this chunk into a
                # normal-format DVE op so scans carry no cross-engine waits.
                nc.vector.tensor_copy(sink[:, :], th[:, 0:1])

                for rr in range(ROWS_PER_CHUNK):
                    r = c * ROWS_PER_CHUNK + rr
                    th_row = th[:, rr * W:(rr + 1) * W]
                    if r == 0:
                        # First row: plain cumsum.  A=BIG, initial=0:
                        # state = min(BIG, state) + th = state + th.
                        nc.vector.tensor_tensor_scan(
                            vpad[:, 1:W + 1], big[:, :], th_row, 0.0,
                            mybir.AluOpType.min, mybir.AluOpType.add,
                        )
                    else:
                        # A[j] = min(V[j], V[j-1])  (vpad[:,0] = BIG handles j=0)
                        a = rpool.tile([BL, W], mybir.dt.float32, tag="A")
                        nc.vector.tensor_tensor(
                            a[:, :], vpad[:, 1:W + 1], vpad[:, 0:W], mybir.AluOpType.min,
                        )
                        nc.vector.tensor_tensor_scan(
                            vpad[:, 1:W + 1], a[:, :], th_row, BIG,
                            mybir.AluOpType.min, mybir.AluOpType.add,
                        )

            nc.sync.dma_start(out=outp[:, :], in_=vpad[:, W:W + 1])

    nc.compile()  # bacc legalization (incl. 1-wait-per-inst event-sem split)
    return nc


def run(images, trace=False):
    """images: np [512, 1, 256, 256] f32 -> (out [512] f32, BassKernelResults)."""
    images = np.ascontiguousarray(np.asarray(images, dtype=np.float32))
    key = "nc"
    if key not in _cache:
        _cache[key] = build()
    nc = _cache[key]
    shards = images.reshape(N_CORES, BL, H * W)
    in_maps = [{"images": np.ascontiguousarray(shards[i])} for i in range(N_CORES)]
    res = run_bass_kernel_spmd(nc, in_maps, list(range(N_CORES)), trace=trace)
    outs = [np.asarray(r["out"]).reshape(BL) for r in res.results]
    return np.concatenate(outs).astype(np.float32), res


def kernel(images):
    out, _ = run(images, trace=False)
    return out
